# revision 47
# baseline (speedup 1.0000x reference)
"""Multi-head self-attention on 8 Trainium2 NeuronCores.

Problem: x:(4,2048,1024) fp32; q = x@Wq, kv = x@Wkv (k,v split), 8 heads of
dim 64, softmax(q k^T / 8) v, concat heads, @Wo + bo -> (4,2048,1024).

Sharding: core c handles batch b=c//2 and head group g=c%2 (4 of 8 heads).
Each core computes its batch's projections restricted to its 4 heads, full
attention for those heads, and a partial output projection y_c = U_norm @ Wo_g.
Host gathers: out[b] = y_{2b} + y_{2b+1} + bo  (the "all-reduce" of the
tensor-parallel head split, done at unshard time).

Device algorithm (per core), PSUM accumulate fp32:
  - q/k/v projections run in fp8e4 DoubleRow mode (0.5 PE cycles/row,
    K=256 per instruction) with residual compensation: the host splits
    x and each W into fp8 value + fp8 residual (W pre-scaled by 32 to
    stay out of e4m3 subnormals), and the GEMM sums x8*W8 + x8*W8r +
    x8r*W8 -- ~0.15% effective error at 75% of the fp16 PE cost.
    The 32*32 q/k scale folds into the softmax exp scale; the v scale
    folds into the softmax normalization.
  - attention per (i-half, head), per j-tile of 128 keys (fp16 operands):
      simT[j,i] = kT_h(j)-tile^T @ qT_h      (PE, K=64)
      expT = exp(SCALE/1024*simT)            (ACT, reads PSUM directly)
      Ut[i,c] += expT(:,i-tile)^T @ [v_h|1]  (PE: expT is the *stationary*
          operand so the moving dim is only 65 wide; accumulated over all
          16 j-tiles in PSUM, 4 i-tiles sharing a bank via pending-zero)
  - normalization: r = (1/32)/Ut[:,64] (DVE Newton), Utn = Ut * r-broadcast
  - Utn[i, (2 heads x 64)d] -> upair[d, i] via DMA-XBAR transpose (idle DMA)
  - y[m,:] = upair_pairs^T @ Wo (K=128 per pair, fp16), drain, DMA (fp16).
Each phase's first sim+exp is emitted inside the previous phase (software
pipelining across the phase boundary), and a credit-based work queue defers
v-projection/attn@v/normalize/final-proj granules into whatever PE slack
each j-slot has, so early phases' projection load never stalls the exp
stream; tail finals drain via the then-idle ACT engine.
"""

import numpy as np

# ---- problem constants (hardcoded per the harness contract) ----
B, N, QDIM = 4, 2048, 1024
HEADS, DIM_MODEL = 8, 512
HEAD_DIM = DIM_MODEL // HEADS  # 64
SCALE = HEAD_DIM ** -0.5  # 0.125
N_CORES = 8
HEADS_PER_CORE = HEADS // 2  # 4
DMC = HEADS_PER_CORE * HEAD_DIM  # 256 per-core model dim slice
WSCALE = 32.0  # host pre-scale on Wq/Wk/Wv so fp8e4 stays in normal range


def build_nc(seq=N, qd=QDIM, nh=HEADS_PER_CORE, hd=HEAD_DIM, dout=QDIM,
             scale=SCALE, ihw=1024, expp_bufs=24, simp_bufs=2,
             uacc_bufs=1, utn_bufs=2, ysb_bufs=6, gw=256):
    """Build the per-core Bass program (same program on all 8 cores)."""
    from contextlib import ExitStack

    import concourse.bass as bass
    import concourse.tile as tile
    from concourse import bacc, mybir

    P = 128
    f16 = mybir.dt.float16
    f32 = mybir.dt.float32
    f8 = mybir.dt.float8e4
    Exp = mybir.ActivationFunctionType.Exp
    DR = mybir.MatmulPerfMode.DoubleRow

    dmc = nh * hd                  # 256
    kt2 = qd // (2 * P)            # 4 double-row contraction steps
    seqt = seq // P                # 16 j-tiles
    mtiles = dmc // P              # 2 (= head pairs)
    n_ih = seq // ihw              # i-halves
    it_per_ih = ihw // P           # i-tiles per half (8)
    IG = 4                         # i-tiles per uacc psum bank tile
    act_scale = scale / (WSCALE * WSCALE)

    nc = bacc.Bacc("TRN2", target_bir_lowering=False, debug=False,
                   num_devices=N_CORES)

    def din(name, shape, dt):
        return nc.dram_tensor(name, shape, dt, kind="ExternalInput").ap()

    # fp8 value+residual packed host-side in SBUF layout:
    # [ki, kd, t, v, cols] with contraction row = kd*256 + t*128 + ki;
    # weights carry the m-tile outermost so each half can be DMA'd alone
    xtp = din("xtp", (P, kt2, 2, 2, seq), f8)
    wp = {k: din(f"w{k}p", (P, mtiles, kt2, 2, 2, P), f8) for k in "qkv"}
    wo = din("wo", (dmc, dout), f16)
    y = nc.dram_tensor("y", (seq, dout), f16, kind="ExternalOutput").ap()

    with tile.TileContext(nc) as tc, ExitStack() as ctx:
        # ---- pools ----
        persist = ctx.enter_context(tc.tile_pool(name="persist", bufs=1))
        expp = ctx.enter_context(tc.tile_pool(name="expp", bufs=expp_bufs))
        utnp = ctx.enter_context(tc.tile_pool(name="utnp", bufs=utn_bufs))
        rows = ctx.enter_context(tc.tile_pool(name="rows", bufs=2))
        ysb = ctx.enter_context(tc.tile_pool(name="ysb", bufs=ysb_bufs))
        simp = ctx.enter_context(
            tc.tile_pool(name="simp", bufs=simp_bufs, space="PSUM"))
        uaccp = ctx.enter_context(
            tc.tile_pool(name="uaccp", bufs=uacc_bufs, space="PSUM"))
        spare = ctx.enter_context(
            tc.tile_pool(name="spare", bufs=2, space="PSUM"))

        # ---- persistent SBUF tensors ----
        xtp_sb = persist.tile([P, kt2, 2, 2, seq], f8)
        wp_sb = {k: persist.tile([P, mtiles, kt2, 2, 2, P], f8,
                                 name=f"w{k}p_sb")
                 for k in "qkv"}
        wo_sb = persist.tile([P, mtiles, dout], f16)
        v_sb = persist.tile([P, seqt, nh, hd + 1], f16)
        qt_sb = persist.tile([P, mtiles, seq], f16)
        kt_sb = persist.tile([P, mtiles, seq], f16)
        upairs = [persist.tile([P, seq], f16, name=f"upair{p}")
                  for p in range(mtiles)]

        # ---- input DMAs, ordered for earliest first-exp ----
        def xt_chunk(c0, cw):
            nc.sync.dma_start(xtp_sb[:, :, :, :, c0:c0 + cw],
                              xtp[:, :, :, :, c0:c0 + cw])

        def w_dma(k, mt):
            nc.sync.dma_start(wp_sb[k][:, mt], wp[k][:, mt])

        # 512-col xt chunks: the DMA model halves throughput below 512B
        # contiguous, so smaller chunks don't arrive any sooner; the mt0
        # weight halves come first to shorten the first-exp critical chain
        w_dma("q", 0)
        xt_chunk(0, 512)
        xt_chunk(512, 512)
        w_dma("k", 0)
        w_dma("v", 0)
        w_dma("v", 1)
        xt_chunk(1024, 1024)
        w_dma("q", 1)
        w_dma("k", 1)
        nc.sync.dma_start(wo_sb[:], wo.rearrange("(t p) n -> p t n", p=P))
        nc.vector.memset(v_sb[:, :, :, hd:hd + 1], 1.0)

        # ---- projection / final-proj granules (fp8 DoubleRow, 3 terms) ----
        TERMS = ((0, 0), (0, 1), (1, 0))  # (x val/res, w val/res) pairs

        def proj_kq(wkey, mt, c0):
            """[128, gw] tile of kT (wkey='k') / qT (wkey='q'), m-tile mt."""
            out_sb = qt_sb if wkey == "q" else kt_sb
            ps = spare.tile([P, 512], f32, tag="ps512", name="ps")
            for ti, (xv, wv_) in enumerate(TERMS):
                for kd in range(kt2):
                    nc.tensor.matmul(
                        ps[:, 0:gw],
                        lhsT=wp_sb[wkey][:, mt, kd, :, wv_, :],
                        rhs=xtp_sb[:, kd, :, xv, c0:c0 + gw],
                        start=(ti == 0 and kd == 0),
                        stop=(ti == 2 and kd == kt2 - 1),
                        perf_mode=DR)
            nc.vector.tensor_copy(out_sb[:, mt, c0:c0 + gw], ps[:, 0:gw])

        def proj_v(jt):
            """v natural layout [128 j, dmc] -> v_sb[:, jt, h, 0:hd].
            One 512B psum region per m-tile inside the same bank, so only
            the very first matmul starts the bank's pending-zero group."""
            ps = spare.tile([P, 512], f32, tag="ps512", name="ps")
            for mt in range(mtiles):
                for ti, (xv, wv_) in enumerate(TERMS):
                    for kd in range(kt2):
                        nc.tensor.matmul(
                            ps[:, mt * P:(mt + 1) * P],
                            lhsT=xtp_sb[:, kd, :, xv, jt * P:(jt + 1) * P],
                            rhs=wp_sb["v"][:, mt, kd, :, wv_, :],
                            start=(mt == 0 and ti == 0 and kd == 0),
                            stop=(mt == mtiles - 1 and ti == 2
                                  and kd == kt2 - 1),
                            perf_mode=DR,
                            skip_group_check=True)
            nc.vector.tensor_copy(
                v_sb[:, jt, :, 0:hd],
                ps[:, 0:dmc].rearrange("p (h d) -> p h d", h=nh))

        tail_ctr = [0]
        ypend = {}

        def final_proj(m, tail=False):
            """y[m*128:(m+1)*128, :] from upairs + wo; mid-kernel y DMAs go
            out in pairs of m-tiles to halve the serialized DMA-issue cost,
            tail ones individually (earlier start beats fewer issues)."""
            if m % 2 == 0:
                ypend[m // 2] = ysb.tile([P, 2, dout], f16, tag="yt",
                                         name="yt")
            yt = ypend[m // 2]
            sl = m % 2
            for ci, n0 in enumerate(range(0, dout, 512)):
                if tail and tail_ctr[0] % 2 == 0:
                    # sim psum is free in the tail; widen drain parallelism
                    ypsb = simp.tile([P, ihw], f32, tag="sim", name="sim")
                    yps = ypsb[:, 0:512]
                else:
                    yps = spare.tile([P, 512], f32, tag="ps512", name="ps")[:]
                tail_ctr[0] += 1
                for pr in range(mtiles):
                    nc.tensor.matmul(
                        yps,
                        lhsT=upairs[pr][:, m * P:(m + 1) * P],
                        rhs=wo_sb[:, pr, n0:n0 + 512],
                        start=(pr == 0), stop=(pr == mtiles - 1))
                if tail and ci == 0:
                    # ACT is idle during the tail; use it for half the drains
                    nc.scalar.mul(yt[:, sl, n0:n0 + 512], yps, 1.0)
                else:
                    nc.vector.tensor_copy(yt[:, sl, n0:n0 + 512], yps)
            if tail:
                nc.sync.dma_start(y[m * P:(m + 1) * P, :], yt[:, sl, :])
                if m % 2 == 1:
                    ypend.pop(m // 2)
            elif m % 2 == 1:
                m0 = m - 1
                nc.sync.dma_start(
                    y[m0 * P:(m0 + 2) * P, :].rearrange(
                        "(two p) n -> p two n", p=P),
                    ypend.pop(m // 2)[:])

        # granule bookkeeping: emit-before-use + metered filler pumping
        emitted = set()

        def emit(key):
            if key in emitted:
                return
            emitted.add(key)
            kind = key[0]
            if kind == 'v':
                proj_v(key[1])
            elif kind in ('k', 'q'):
                proj_kq(kind, key[1], key[2])
            else:
                final_proj(key[1])

        fill_order = []
        fill_order += [('q', 0, 0), ('q', 0, 256), ('k', 0, 0), ('v', 0),
                       ('q', 0, 512), ('q', 0, 768), ('k', 0, 256)]
        fill_order += [('v', 1), ('v', 2), ('v', 3), ('k', 0, 512),
                       ('v', 4), ('v', 5), ('k', 0, 768)]
        fill_order += [('v', jt) for jt in range(6, 10)]
        fill_order += [('k', 0, 1024), ('k', 0, 1280)]
        fill_order += [('v', jt) for jt in range(10, 14)]
        fill_order += [('k', 0, 1536), ('k', 0, 1792)]
        fill_order += [('v', 14), ('v', 15)]
        fill_order += [('k', 1, c) for c in range(0, seq, gw)]
        fill_order += [('q', 1, 0), ('q', 1, 256), ('q', 1, 512),
                       ('q', 1, 768)]
        fill_order += [('q', 0, c) for c in range(1024, 2048, gw)]
        fill_order += [('q', 1, c) for c in range(1024, 2048, gw)]

        def emit_uacc(h, jt, expt, uacc, first, last=False):
            for it in range(it_per_ih):
                g, sl = it // IG, it % IG
                nc.tensor.matmul(
                    uacc[g][:, sl:sl + 1, :],
                    lhsT=expt[:, it * P:(it + 1) * P],
                    rhs=v_sb[:, jt, h, :],
                    start=(first and sl == 0),
                    stop=(last and sl == IG - 1),
                    skip_group_check=True)

        # ---- attention phases (i-half major, head minor) ----
        phases = [(ih, h) for ih in range(n_ih) for h in range(nh)]
        utn_tiles = {}
        for ih in range(n_ih):
            for pair in range(mtiles):
                utn_tiles[(ih, pair)] = utnp.tile(
                    [P, it_per_ih, 2 * hd], f16, tag=f"utn{pair}",
                    name=f"utn{pair}")
        state = {}

        def emit_sim_exp(ih, h, jt):
            mt, hb, i0 = h // 2, (h % 2) * hd, ih * ihw
            sim = simp.tile([P, ihw], f32, tag="sim", name="sim")
            for c0 in range(0, ihw, 512):
                nc.tensor.matmul(
                    sim[:, c0:c0 + 512],
                    lhsT=kt_sb[hb:hb + hd, mt, jt * P:(jt + 1) * P],
                    rhs=qt_sb[hb:hb + hd, mt, i0 + c0:i0 + c0 + 512],
                    start=True, stop=True)
            expt = expp.tile([P, ihw], f16, tag="expt", name="expt")
            nc.scalar.activation(expt[:], sim[:], Exp, scale=act_scale)
            return expt

        def emit_normalize(pi, uacc):
            """r = 1/s (native DVE reciprocal, exact), then Utn =
            (Ut / WSCALE) * r fused (folds away the v fp8 scale; r
            broadcast over d); transposes + tail finals chase each group."""
            ih, h = phases[pi]
            hcol = (h % 2) * hd
            pair = h // 2
            utn = utn_tiles[(ih, pair)]
            last_phase = pi == len(phases) - 1
            r_sb = rows.tile([P, it_per_ih, 1], f32, tag="r", name="r_sb")
            for g in range(it_per_ih // IG):
                nc.vector.reciprocal(r_sb[:, g * IG:(g + 1) * IG, :],
                                     uacc[g][:, :, hd:hd + 1])
            for g in range(it_per_ih // IG):
                nc.vector.scalar_tensor_tensor(
                    utn[:, g * IG:(g + 1) * IG, hcol:hcol + hd],
                    uacc[g][:, :, 0:hd],
                    1.0 / WSCALE,
                    r_sb[:, g * IG:(g + 1) * IG, :].broadcast_to(
                        [P, IG, hd]),
                    op0=mybir.AluOpType.mult, op1=mybir.AluOpType.mult)
                if h % 2 == 1:
                    for it in range(g * IG, (g + 1) * IG):
                        m = ih * it_per_ih + it
                        nc.sync.dma_start_transpose(
                            upairs[pair][:, m * P:(m + 1) * P],
                            utn[:, it:it + 1, :])
                        if last_phase:
                            final_proj(m, tail=True)
            if h == nh - 1 and ih < n_ih - 1:
                fill_order.extend(
                    ('y', ih * it_per_ih + it) for it in range(it_per_ih))

        # Deferred-work queue: uacc GEMMs (plus their v-granule inputs and
        # the closing normalize) may lag their exps by several j-slots, so
        # early phases' projection load spills into later phases' PE slack
        # instead of stalling ACT. Items: ('v', jt) | ('uacc', pi, jt, expt)
        # | ('norm', pi). Costs are approximate full-speed PE ns.
        from collections import deque
        work = deque()
        SLOT_SLACK = 540.0   # ACT 1038 - sim 427 - margin
        KCOST, VCOST, UCOST, YCOST = 640.0, 640.0, 220.0, 870.0
        credit = [0.0]

        def drain(extra=0.0):
            credit[0] = min(credit[0] + extra, 2000.0)
            while True:
                if work:
                    kind = work[0][0]
                    cost = (0.0 if kind == 'norm' else
                            0.0 if kind == 'v' and work[0] in emitted else
                            VCOST if kind == 'v' else UCOST)
                    if cost > credit[0]:
                        return
                    item = work.popleft()
                    if kind == 'v':
                        emit(item)
                    elif kind == 'uacc':
                        _, pi_, jt_, expt_ = item
                        emit_uacc(phases[pi_][1], jt_, expt_,
                                  state[pi_][0], first=(jt_ == 0),
                                  last=(jt_ == seqt - 1))
                    else:
                        emit_normalize(item[1], state.pop(item[1])[0])
                    credit[0] -= cost
                else:
                    nxt = next((k for k in fill_order if k not in emitted),
                               None)
                    if nxt is None or credit[0] < (
                            YCOST if nxt[0] == 'y' else KCOST):
                        return
                    emit(nxt)
                    credit[0] -= YCOST if nxt[0] == 'y' else KCOST

        def prestart(pi):
            """Emit phase pi's inputs + its first sim/exp (called from the
            tail of phase pi-1 so ACT never idles across the boundary)."""
            ih, h = phases[pi]
            mt = h // 2
            for c0 in range(ih * ihw, ih * ihw + ihw, gw):
                emit(('q', mt, c0))
            emit(('k', mt, 0))
            uacc = [uaccp.tile([P, IG, hd + 1], f32, tag=f"uacc{g}",
                               name=f"uacc{g}")
                    for g in range(it_per_ih // IG)]
            expt = emit_sim_exp(ih, h, 0)
            state[pi] = (uacc, expt)
            work.append(('v', 0))
            work.append(('uacc', pi, 0, expt))

        def run_phase(pi):
            ih, h = phases[pi]
            mt = h // 2
            last_phase = pi == len(phases) - 1

            for jt in range(1, seqt):
                before = ('k', mt, (jt * P) // gw * gw)
                kc = 0.0
                if before not in emitted:
                    emit(before)
                    kc = KCOST
                expt = emit_sim_exp(ih, h, jt)
                work.append(('v', jt))
                work.append(('uacc', pi, jt, expt))
                drain(SLOT_SLACK - kc)
            work.append(('norm', pi))
            if not last_phase:
                prestart(pi + 1)
            else:
                # flush everything that remains
                credit[0] = 1e9
                drain()

        prestart(0)
        for pi in range(len(phases)):
            run_phase(pi)
        credit[0] = 1e9
        drain()

    nc.compile()
    return nc


_NC_CACHE = {}


def _get_nc():
    if "nc" not in _NC_CACHE:
        _NC_CACHE["nc"] = build_nc()
    return _NC_CACHE["nc"]


def _q8(a, f8):
    """fp8 value + fp8 residual split (both rounded-to-nearest)."""
    v = a.astype(f8)
    r = (a - v.astype(np.float32)).astype(f8)
    return v, r


def _prep_core_inputs(x, Wq, Wkv, Wo):
    """Host-side shard + layout prep: per-core fp8(+residual) slices."""
    import ml_dtypes
    f8 = ml_dtypes.float8_e4m3

    def pack(a):
        """(1024, cols) -> (128 ki, 4 kd, 2 t, 2 v, cols) fp8 val/resid."""
        ar = np.ascontiguousarray(a).reshape(4, 2, 128, a.shape[1])
        v8, r8 = _q8(ar, f8)
        return np.ascontiguousarray(
            np.stack([v8, r8], axis=3).transpose(2, 0, 1, 3, 4))

    def packw(a):
        """(1024, 256) -> (128 ki, 2 mt, 4 kd, 2 t, 2 v, 128) fp8."""
        p = pack(a)  # (128, 4, 2, 2, 256)
        p = p.reshape(128, 4, 2, 2, 2, 128)  # split d -> (mt, 128)
        return np.ascontiguousarray(p.transpose(0, 4, 1, 2, 3, 5))

    in_maps = []
    for c in range(N_CORES):
        b, g = c // 2, c % 2
        s = slice(g * DMC, (g + 1) * DMC)
        in_maps.append({
            "xtp": pack(x[b].T),
            "wqp": packw(Wq[:, s] * WSCALE),
            "wkp": packw(Wkv[:, g * DMC:(g + 1) * DMC] * WSCALE),
            "wvp": packw(
                Wkv[:, DIM_MODEL + g * DMC:DIM_MODEL + (g + 1) * DMC]
                * WSCALE),
            "wo": np.ascontiguousarray(Wo[s, :]).astype(np.float16),
        })
    return in_maps


def kernel(x, Wq, Wkv, Wo, bo):
    from concourse import bass_utils

    x = np.asarray(x, dtype=np.float32)
    Wq = np.asarray(Wq, dtype=np.float32)
    Wkv = np.asarray(Wkv, dtype=np.float32)
    Wo = np.asarray(Wo, dtype=np.float32)
    bo = np.asarray(bo, dtype=np.float32)

    nc = _get_nc()
    in_maps = _prep_core_inputs(x, Wq, Wkv, Wo)
    res = bass_utils.run_bass_kernel_spmd(nc, in_maps,
                                          core_ids=list(range(N_CORES)))
    out = np.empty((B, N, QDIM), dtype=np.float32)
    for b in range(B):
        out[b] = (res.results[2 * b]["y"].astype(np.float32)
                  + res.results[2 * b + 1]["y"].astype(np.float32) + bo)
    return out
